# revision 43
# baseline (speedup 1.0000x reference)
"""Bahdanau additive attention on TRN2, data-parallel over batch on 8 NeuronCores.

Reference computation (per batch b):
    pre[s, :]  = W1 @ hs[s, b, :] + b1 + W2 @ hidden[b, :] + b2      # (S, H)
    energy[s]  = v . tanh(pre[s, :])                                  # (S,)
    energy     = where(mask[s, b], energy, -1e10)
    attn       = softmax(energy over s)
    ctx[b, :]  = sum_s attn[s] * hs[s, b, :]                          # (H,)

Key optimizations over a dense implementation (351us -> ~155us on 8xTRN2):
  - Mask compaction (host side): masked-out sequence positions contribute
    exactly 0 to the output (exp(-1e10 - max) underflows to 0.0 in f32, same
    as the reference), so each batch's unmasked rows are gathered into a
    compact buffer of length SC = roundup128(max unmasked count) ~= 0.5*S.
    All energy/context work scales with SC instead of S.
  - fp16 operands for the big W1 matmul: same 1 cycle/row PE rate as f32r,
    but half the HBM traffic and fast-weight-load on the stationary.
  - The v . tanh() reduction runs as 8 DVE mult-accumulate ops (per-partition
    scalar v) + a single ones-vector matmul, instead of 8 PE matmuls.
  - W1 is packed m-major on the host and interleaved with the first hst
    block on the ordered SWDGE queue, so the first PSUM group starts after
    ~0.5 MB of DMA instead of 5 MB.
  - The -1e10 masking is folded into the PSUM->SBUF energy copy, and block
    maxima are reduced per-block, both hidden under the next block's MMs.
  - attn is pre-scaled by 1/Z before the transposes, so the context PSUM is
    final and drains with two copies + one strided-partition DMA.
  - Context matmuls (M=1) for the 4 batches run CONCURRENTLY in the PE
    array via column tiling (tile_position=(0,32b)); batches 0-2 execute
    inside the last batch's softmax latency shadow.
  - The q = W2 @ hidden matmuls are emitted behind the first energy block so
    the PE never queues on the 2 MiB W2 load.

Per-core layout (batch-sharded, 4 batches per core):
  - hst shard (BL, H, SC) fp16, h-major: the big matmul streams [h=128p, s]
    tiles; preT comes out [k=128p, s] in PSUM so the q/bias add is a
    per-partition activation bias.
  - hsn shard (SC, BL, H) bf16, s-major for the context matmul.
"""

import sys
from contextlib import ExitStack

import numpy as np
import ml_dtypes

if "/opt/trn_rl_repo" not in sys.path:
    sys.path.append("/opt/trn_rl_repo")

import concourse.bass as bass
import concourse.bacc as bacc
import concourse.mybir as mybir
import concourse.tile as tile
from concourse import bass_utils

S, B, H = 2048, 32, 1024
NCORES = 8
BL = B // NCORES  # local batches per core
HK = H // 128     # 128-partition chunks of H

F32 = mybir.dt.float32
F32R = mybir.dt.float32r
F16 = mybir.dt.float16
U8 = mybir.dt.uint8
BF16 = mybir.dt.bfloat16
AF = mybir.ActivationFunctionType
AX = mybir.AxisListType
ALU = mybir.AluOpType

_CACHE = {}


def _blocks(sc):
    """Sigma-block widths: 512s plus one 128-granular remainder."""
    out = [512] * (sc // 512)
    if sc % 512:
        out.append(sc % 512)
    return out


def _emit(tc, aps, sc):
    nc = tc.nc
    ctx = aps["ctx_stack"]
    hst, hsn, w1m, w2t, bvt, hidr, masku, ctx_out = (
        aps["hst"], aps["hsn"], aps["w1m"], aps["w2t"],
        aps["bvt"], aps["hidr"], aps["masku"], aps["ctx"],
    )
    SIGB = _blocks(sc)
    NSIG = len(SIGB)
    TT = sc // 128  # context s-tiles per batch

    def pool(name, bufs, space="SBUF"):
        return ctx.enter_context(tc.tile_pool(name=name, bufs=bufs, space=space))

    p_hst = pool("hst", 5)
    p_w1 = pool("w1", 1)
    p_w2c = pool("w2c", 8)
    p_small = pool("small", 1)
    p_hsn = pool("hsn", BL + 1)
    p_tanh = pool("tanh", 3)
    p_part = pool("part", 2)
    p_eall = pool("eall", 2)
    p_mask = pool("mask", 1)
    p_ctxs = pool("ctxs", 1)
    p_attnT = pool("attnT", BL)
    p_sc = pool("sc", 16)

    pp_pre = pool("ppre", 4, space="PSUM")
    pp_en = pool("pen", 1, space="PSUM")
    pp_tr = pool("ptr", 1, space="PSUM")
    pp_ctx = pool("pctx", 2, space="PSUM")

    # ---------------- setup ----------------
    ident = p_small.tile([1, 1], F32, tag="ident")
    nc.gpsimd.memset(ident[:], 1.0)
    # warm the ACT table set (exp_and_others covers tanh+exp) during DMA wait
    dummy = p_small.tile([1, 1], F32, tag="dummy")
    nc.scalar.activation(dummy[:], ident[:], AF.Tanh)

    # bvt packs [bsum(b1+b2) | vt | eye4 | ones] as (128, 2*HK+5)
    bvt_sb = p_small.tile([128, 2 * HK + 5], F32R, tag="bvt")
    nc.sync.dma_start(bvt_sb[:], bvt[:])
    bsum_sb = bvt_sb[:, 0:HK].bitcast(F32)
    vt_f32 = bvt_sb[:, HK:2 * HK].bitcast(F32)
    eye4 = bvt_sb[0:4, 2 * HK:2 * HK + 4].bitcast(F32)
    ones_col = bvt_sb[:, 2 * HK + 4:2 * HK + 5]  # F32R ones for the v-dot MM
    # hidden (pre-swizzled on host to [p, 4k+b]), fp16 to match the W2 rows
    hid_sb = p_small.tile([128, BL * HK], F16, tag="hidr")
    nc.sync.dma_start(hid_sb[:], hidr[:])

    def load_hst(dst, b, off, W, swdge=False):
        if swdge:
            # one SWDGE DMA: spread over all 16 engines, strictly ordered in
            # the gpsimd queue -- used for the first blocks so the startup
            # stream arrives in consumption order.
            nc.gpsimd.dma_start(
                dst.rearrange("p (k j) -> p k j", k=HK),
                hst[b, :, off:off + W].rearrange("(k p) j -> p k j", p=128),
            )
            return
        # 8 per-k DMAs: each HWDGE dma_start lands on one DMA engine, so
        # splitting is what buys transfer parallelism.
        for k in range(HK):
            nc.sync.dma_start(
                dst[:, W * k:W * (k + 1)],
                hst[b, 128 * k:128 * (k + 1), off:off + W],
            )

    # First hst block + W1 interleaved on the gpsimd queue: SWDGE spreads
    # each DMA across all 16 engines, so these land in consumption order
    # and the first matmul group starts ~1us in.
    hst_first = p_hst.tile([128, HK * SIGB[0]], F16, tag="hst", name="hst_first")
    w1_sb = p_w1.tile([128, HK * H], F16, tag="w1")

    def w1_load(m):
        nc.gpsimd.dma_start(w1_sb[:, HK * 128 * m:HK * 128 * (m + 1)], w1m[m])

    w1_load(0)
    for k in range(4):
        nc.gpsimd.dma_start(
            hst_first[:, SIGB[0] * k:SIGB[0] * (k + 1)],
            hst[0, 128 * k:128 * (k + 1), 0:SIGB[0]],
        )
    w1_load(1)
    for k in range(4, HK):
        nc.gpsimd.dma_start(
            hst_first[:, SIGB[0] * k:SIGB[0] * (k + 1)],
            hst[0, 128 * k:128 * (k + 1), 0:SIGB[0]],
        )
    for m in range(2, HK):
        w1_load(m)

    # all four batch masks in one row
    mask_all = p_mask.tile([1, BL * sc], U8, tag="mask")
    nc.sync.dma_start(mask_all[:], masku[:])

    # W2 rows for the q matmul (scalar queue: parallel issue with hst/w1;
    # ACT is idle until the first tanh which needs qt anyway)
    w2rs = []
    for k in range(HK):
        w2r = p_w2c.tile([128, H], F16, tag="w2c", name=f"w2r{k}")
        nc.scalar.dma_start(w2r[:], w2t[128 * k:128 * (k + 1), :])
        w2rs.append(w2r)

    # qT[h_out, b] = sum_hin W2[h_out, hin] * hidden[b, hin] + b1 + b2.
    # Emitted AFTER the first energy block's matmuls (via q_hook) so the main
    # stream doesn't queue behind the 2 MiB W2 load.
    qt_sb = p_small.tile([128, BL * HK], F32, tag="qt")
    qn_sb = p_small.tile([BL, H], F32, tag="qnat")

    def emit_q():
        for n in range(2):
            pq = pp_ctx.tile([BL, 512], F32, tag="pctx", name=f"pq{n}")
            for k in range(HK):
                nc.tensor.matmul(
                    pq[:],
                    lhsT=hid_sb[:, BL * k:BL * (k + 1)],
                    rhs=w2rs[k][:, 512 * n:512 * (n + 1)],
                    start=(k == 0), stop=(k == HK - 1),
                )
            nc.vector.tensor_copy(qn_sb[:, 512 * n:512 * (n + 1)], pq[:])
        # transpose q to [h_out partition, b] and fold in b1+b2
        ptrq = pp_tr.tile([128, BL * HK], F32, tag="ptr", name="ptrq")
        for m in range(HK):
            nc.tensor.transpose(
                ptrq[:, BL * m:BL * (m + 1)], qn_sb[:, 128 * m:128 * (m + 1)], eye4
            )
        for m in range(HK):
            nc.vector.tensor_scalar_add(
                qt_sb[:, BL * m:BL * (m + 1)], ptrq[:, BL * m:BL * (m + 1)],
                bsum_sb[:, m:m + 1],
            )

    eall_t = {}
    bmax_t = {}
    attnT_t = {}
    rz_t = {}

    # ------------- pass 1: energies for one (batch, sigma-block) -------------
    def p1_block(b, c, first_tile=None, q_hook=None):
        W = SIGB[c]
        off = 512 * c  # block start (all non-final blocks are 512 wide)
        if c == 0:
            eall_t[b] = p_eall.tile([1, sc], F32, tag="eall", name=f"eall{b}")
            bmax_t[b] = p_sc.tile([1, NSIG], F32, tag="bmax", name=f"bmax{b}")
        eall = eall_t[b]
        if first_tile is not None:
            hst_c = first_tile
        else:
            hst_c = p_hst.tile([128, HK * W], F16, tag="hst", name=f"hst_{b}_{c}")
            # the first two loaded blocks ride the ordered SWDGE queue right
            # behind W1/hst_first; later blocks use parallel HWDGE DMAs
            load_hst(hst_c[:], b, off, W, swdge=(b, c) <= (1, 0))
        part = p_part.tile([128, W], F32R, tag="part", name=f"part_{b}_{c}")
        ppres = {}

        def emit_group(m):
            ppre = pp_pre.tile([128, W], F32, tag="ppre", name=f"ppre_{b}_{c}_{m}")
            for k in range(HK):
                nc.tensor.matmul(
                    ppre[:],
                    lhsT=w1_sb[:, (m * HK + k) * 128:(m * HK + k) * 128 + 128],
                    rhs=hst_c[:, W * k:W * (k + 1)],
                    start=(k == 0), stop=(k == HK - 1),
                )
            ppres[m] = ppre

        def emit_act(m):
            th = p_tanh.tile([128, W], F32, tag="tanh", name=f"th_{b}_{c}_{m}")
            nc.scalar.activation(
                th[:], ppres.pop(m), AF.Tanh,
                bias=qt_sb[:, BL * m + b:BL * m + b + 1], scale=1.0,
            )
            # v-weighted accumulate on DVE (frees the PE for the next m-group)
            if m == 0:
                nc.vector.tensor_scalar_mul(part[:], th[:], vt_f32[:, 0:1])
            else:
                nc.vector.scalar_tensor_tensor(
                    part[:], th[:], vt_f32[:, m:m + 1], part[:],
                    op0=ALU.mult, op1=ALU.add,
                )

        if q_hook is not None:
            # first block: matmul groups go out ahead of the q computation so
            # the PE starts on them as soon as W1/hst land; the tanh stream
            # (which reads qt) is emitted after the hook.
            for m in range(4):
                emit_group(m)
            q_hook()
            for m in range(4, HK):
                emit_act(m - 4)
                emit_group(m)
            for m in range(4, HK):
                emit_act(m)
        else:
            for m in range(HK):
                emit_group(m)
                emit_act(m)
        # partition-sum of the v-weighted tanh: single ones-vector matmul
        pen = pp_en.tile([1, W], F32, tag="pen", name=f"pen_{b}_{c}")
        nc.tensor.matmul(pen[:], lhsT=ones_col, rhs=part[:], start=True, stop=True)
        # fold the -1e10 mask into the PSUM->SBUF copy, then per-block max
        # (both hide under the next block's matmuls)
        nc.vector.scalar_tensor_tensor(
            eall[:, off:off + W], mask_all[:, b * sc + off:b * sc + off + W],
            -1e10, pen[:], op0=ALU.mult, op1=ALU.add,
        )
        nc.vector.reduce_max(
            bmax_t[b][:, c:c + 1], eall[:, off:off + W], axis=AX.X
        )

    # ------------- masked softmax, split so it interleaves with pass 1 ------
    def sm_pre(b):
        """exposed part: global max, exp, Z, 1/Z. No PE work."""
        eall = eall_t[b]
        bmax = bmax_t.pop(b)
        negmax = p_sc.tile([1, 1], F32, tag="negmax", name=f"negmax{b}")
        nc.vector.reduce_max(negmax[:], bmax[:], axis=AX.X, negate=True)
        zs = p_sc.tile([1, 1], F32, tag="zs", name=f"zs{b}")
        nc.scalar.activation(eall[:], eall[:], AF.Exp, bias=negmax[:], scale=1.0, accum_out=zs[:])
        rz = p_sc.tile([1, 1], F32, tag="rz", name=f"rz{b}")
        nc.vector.reciprocal(rz[:], zs[:])
        rz_t[b] = rz

    def sm_tr(b):
        """PE part: TT tiny transposes of attn into [s-partition, 1] layout.
        The transpose 'identity' scalar is 1/Z, so attn comes out normalized
        and no post-context rescale is needed."""
        eall = eall_t.pop(b)
        rz = rz_t.pop(b)
        # scale halves separately so the first transposes start half-early
        half = (TT // 2) * 128
        nc.vector.tensor_scalar_mul(eall[:, 0:half], eall[:, 0:half], rz[:])
        nc.vector.tensor_scalar_mul(eall[:, half:sc], eall[:, half:sc], rz[:])
        ptr = pp_tr.tile([128, TT], F32, tag="ptr", name=f"ptr{b}")
        for cc in range(TT):
            nc.tensor.transpose(ptr[:, cc:cc + 1], eall[:, 128 * cc:128 * (cc + 1)], ident[:])
        att = p_attnT.tile([128, TT], BF16, tag="attnT", name=f"attnT{b}")
        nc.vector.tensor_copy(att[:], ptr[:])
        attnT_t[b] = att

    # ------------- pass 2: context, all batches column-packed -------------
    hsn_tiles = {}

    def p2_load(b):
        hsn_b = p_hsn.tile([128, TT * H], BF16, tag="hsn", name=f"hsn_{b}")
        nc.gpsimd.dma_start(
            hsn_b[:].rearrange("p (t h) -> p t h", t=TT),
            hsn[:, b, :].rearrange("(t p) h -> p t h", p=128),
        )
        hsn_tiles[b] = hsn_b

    pcs_t = []

    def p2_mm(batches):
        # batches run concurrently in the PE array via column tiling: each
        # M=1 matmul occupies one 32-column group with its own stream.
        if not pcs_t:
            for n in range(2):
                pc = pp_ctx.tile([128, 512], F32, tag="pctx", name=f"pcn{n}")
                # zero the unwritten rows so the full-width drain copy is
                # clean; runs early on DVE, hidden under pass-1 matmuls
                nc.vector.memset(pc[:], 0.0)
                pcs_t.append(pc)
        for n in range(2):
            for t in range(TT):
                for b in batches:
                    nc.tensor.matmul(
                        pcs_t[n][32 * b:32 * b + 1, :],
                        lhsT=attnT_t[b][:, t:t + 1],
                        rhs=hsn_tiles[b][:, t * H + 512 * n:t * H + 512 * n + 512],
                        start=(t == 0), stop=(t == TT - 1),
                        tile_position=(0, 32 * b),
                        skip_group_check=True,
                    )

    def p2_finish():
        # context is already normalized (attn carried 1/Z): drain the two
        # halves on different engines; per-half strided-partition DMAs so the
        # n=0 half ships while the n=1 copy still runs
        cs_all = p_ctxs.tile([128, H], F32, tag="ctxs")
        nc.vector.tensor_copy(cs_all[:, 0:512], pcs_t[0][:])
        nc.sync.dma_start(ctx_out[:, 0:512], cs_all[::32, 0:512])
        nc.scalar.activation(cs_all[:, 512:H], pcs_t[1][:], AF.Copy)
        nc.sync.dma_start(ctx_out[:, 512:H], cs_all[::32, 512:H])
        for b in range(BL):
            attnT_t.pop(b)
            hsn_tiles.pop(b)

    # ------------- schedule -------------
    # sm(b) pieces interleave into the middle of batch b+1's PE stream so the
    # softmax chain latency hides behind matmuls; the packed context phase
    # runs once at the end when all four attn vectors are ready.
    p1_block(0, 0, first_tile=hst_first, q_hook=emit_q)
    for c in range(1, NSIG):
        p1_block(0, c)
    for b in range(1, BL):
        p1_block(b, 0)
        sm_pre(b - 1)
        p2_load(b - 1)  # hsn loads stay clear of the startup DMA burst
        if NSIG > 1:
            p1_block(b, 1)
        sm_tr(b - 1)
        for c in range(2, NSIG):
            p1_block(b, c)
    p2_load(BL - 1)
    # batches 0..BL-2 run their (packed) context matmuls while the last
    # batch's softmax chain drains; the last batch joins afterwards.
    p2_mm(list(range(BL - 1)))
    sm_pre(BL - 1)
    sm_tr(BL - 1)
    p2_mm([BL - 1])
    p2_finish()


def build_program(sc):
    if sc in _CACHE:
        return _CACHE[sc]
    nc = bacc.Bacc("TRN2", target_bir_lowering=False, debug=False, enable_asserts=False)
    aps = {
        "hst": nc.dram_tensor("hst", (BL, H, sc), F16, kind="ExternalInput").ap(),
        "hsn": nc.dram_tensor("hsn", (sc, BL, H), BF16, kind="ExternalInput").ap(),
        "w1m": nc.dram_tensor("w1m", (HK, 128, H), F16, kind="ExternalInput").ap(),
        "w2t": nc.dram_tensor("w2t", (H, H), F16, kind="ExternalInput").ap(),
        "bvt": nc.dram_tensor("bvt", (128, 2 * HK + 5), F32R, kind="ExternalInput").ap(),
        "hidr": nc.dram_tensor("hidr", (128, BL * HK), F16, kind="ExternalInput").ap(),
        "masku": nc.dram_tensor("masku", (1, BL * sc), U8, kind="ExternalInput").ap(),
        "ctx": nc.dram_tensor("ctx", (BL, H), F32, kind="ExternalOutput").ap(),
    }
    with tile.TileContext(nc) as tc:
        with ExitStack() as stack:
            aps["ctx_stack"] = stack
            _emit(tc, aps, sc)
    nc.compile()
    _CACHE[sc] = nc
    return nc


def prep_in_maps(inputs):
    hidden = np.ascontiguousarray(np.asarray(inputs["hidden"], dtype=np.float32))
    hs = np.asarray(inputs["hidden_sequence"], dtype=np.float32)
    masks = np.asarray(inputs["input_masks"]).astype(bool)
    w1 = np.asarray(inputs["W1"], dtype=np.float32)
    w2t = np.ascontiguousarray(np.asarray(inputs["W2"], dtype=np.float32).T.astype(np.float16))
    b1 = np.asarray(inputs["b1"], dtype=np.float32)
    b2 = np.asarray(inputs["b2"], dtype=np.float32)
    v = np.asarray(inputs["v"], dtype=np.float32)

    # ---- mask compaction: per batch, gather unmasked rows ----
    counts = masks.sum(axis=0)
    if counts.min() == 0:
        # all-masked batch: softmax degenerates to uniform over the FULL S,
        # so compaction would change the result. Fall back to identity order.
        sc = S
        idx_list = [np.arange(S)] * B
    else:
        sc = max(128, int(-(-int(counts.max()) // 128) * 128))
        idx_list = [np.nonzero(masks[:, b])[0] for b in range(B)]

    # compact hs and the inverted mask
    hs_c = np.zeros((B, sc, H), dtype=np.float32)
    minv = np.ones((B, sc), dtype=np.uint8)
    for b in range(B):
        idx = idx_list[b]
        n = len(idx)
        hs_c[b, :n] = hs[idx[:n] if n else idx, b, :]
        if sc == S and counts.min() == 0:
            # identity order: apply the original mask
            hs_c[b] = hs[:, b, :]
            minv[b] = (~masks[:, b]).astype(np.uint8)
        else:
            minv[b, :n] = 0

    # [bsum | vt | eye4 | ones] packed as (128, 2*HK+5)
    ey = np.zeros((128, 4), dtype=np.float32)
    ey[0:4, 0:4] = np.eye(4, dtype=np.float32)
    ones = np.ones((128, 1), dtype=np.float32)
    bvt_base = np.concatenate(
        [(b1 + b2).reshape(HK, 128).T, v.reshape(HK, 128).T, ey, ones], axis=1
    )

    # W1 packed m-major: w1m[m][p, k*128+j] = W1[128m+j, 128k+p]
    w1m = np.ascontiguousarray(
        w1.reshape(HK, 128, HK, 128).transpose(0, 3, 2, 1).reshape(HK, 128, H)
    ).astype(np.float16)

    in_maps = []
    for ci in range(NCORES):
        g = slice(BL * ci, BL * (ci + 1))
        blk = hs_c[g]  # (BL, sc, H)
        hg = hidden[0, g, :]  # (BL, H)
        # hidr[p, BL*k + b] = hidden[b, 128k + p]
        hidr = np.ascontiguousarray(
            hg.T.reshape(HK, 128, BL).transpose(1, 0, 2).reshape(128, HK * BL).astype(np.float16)
        )
        in_maps.append({
            "hst": np.ascontiguousarray(blk.transpose(0, 2, 1)).astype(np.float16),
            "hsn": np.ascontiguousarray(blk.transpose(1, 0, 2).astype(ml_dtypes.bfloat16)),
            "w1m": w1m,
            "w2t": w2t,
            "bvt": np.ascontiguousarray(bvt_base),
            "hidr": hidr,
            "masku": np.ascontiguousarray(minv[g]).reshape(1, BL * sc),
        })
    return sc, in_maps


def _gather(res):
    out = np.concatenate([res.results[i]["ctx"] for i in range(NCORES)], axis=0)
    return out[None].astype(np.float32)


def kernel(**inputs):
    sc, in_maps = prep_in_maps(inputs)
    nc = build_program(sc)
    res = bass_utils.run_bass_kernel_spmd(nc, in_maps, list(range(NCORES)))
    return _gather(res)


def run_traced(inputs, trace=True):
    """test.py entry: returns (BassKernelResults, full output)."""
    sc, in_maps = prep_in_maps(inputs)
    nc = build_program(sc)
    res = bass_utils.run_bass_kernel_spmd(
        nc, in_maps, list(range(NCORES)), trace=trace
    )
    return res, _gather(res)


if __name__ == "__main__":
    build_program(1152)
    print("program built OK")


# revision 44
# speedup vs baseline: 1.0190x; 1.0190x over previous
"""Bahdanau additive attention on TRN2, data-parallel over batch on 8 NeuronCores.

Reference computation (per batch b):
    pre[s, :]  = W1 @ hs[s, b, :] + b1 + W2 @ hidden[b, :] + b2      # (S, H)
    energy[s]  = v . tanh(pre[s, :])                                  # (S,)
    energy     = where(mask[s, b], energy, -1e10)
    attn       = softmax(energy over s)
    ctx[b, :]  = sum_s attn[s] * hs[s, b, :]                          # (H,)

Key optimizations over a dense implementation (351us -> ~155us on 8xTRN2):
  - Mask compaction (host side): masked-out sequence positions contribute
    exactly 0 to the output (exp(-1e10 - max) underflows to 0.0 in f32, same
    as the reference), so each batch's unmasked rows are gathered into a
    compact buffer of length SC = roundup128(max unmasked count) ~= 0.5*S.
    All energy/context work scales with SC instead of S.
  - fp16 operands for the big W1 matmul: same 1 cycle/row PE rate as f32r,
    but half the HBM traffic and fast-weight-load on the stationary.
  - The v . tanh() reduction runs as 8 DVE mult-accumulate ops (per-partition
    scalar v) + a single ones-vector matmul, instead of 8 PE matmuls.
  - W1 is packed m-major on the host and interleaved with the first hst
    block on the ordered SWDGE queue, so the first PSUM group starts after
    ~0.5 MB of DMA instead of 5 MB.
  - The -1e10 masking is folded into the PSUM->SBUF energy copy, and block
    maxima are reduced per-block, both hidden under the next block's MMs.
  - attn is pre-scaled by 1/Z before the transposes, so the context PSUM is
    final and drains with two copies + one strided-partition DMA.
  - Context matmuls (M=1) for the 4 batches run CONCURRENTLY in the PE
    array via column tiling (tile_position=(0,32b)); batches 0-2 execute
    inside the last batch's softmax latency shadow.
  - The q = W2 @ hidden matmuls are emitted behind the first energy block so
    the PE never queues on the 2 MiB W2 load.

Per-core layout (batch-sharded, 4 batches per core):
  - hst shard (BL, H, SC) fp16, h-major: the big matmul streams [h=128p, s]
    tiles; preT comes out [k=128p, s] in PSUM so the q/bias add is a
    per-partition activation bias.
  - hsn shard (SC, BL, H) bf16, s-major for the context matmul.
"""

import sys
from contextlib import ExitStack

import numpy as np
import ml_dtypes

if "/opt/trn_rl_repo" not in sys.path:
    sys.path.append("/opt/trn_rl_repo")

import concourse.bass as bass
import concourse.bacc as bacc
import concourse.mybir as mybir
import concourse.tile as tile
from concourse import bass_utils

S, B, H = 2048, 32, 1024
NCORES = 8
BL = B // NCORES  # local batches per core
HK = H // 128     # 128-partition chunks of H

F32 = mybir.dt.float32
F32R = mybir.dt.float32r
F16 = mybir.dt.float16
U8 = mybir.dt.uint8
BF16 = mybir.dt.bfloat16
AF = mybir.ActivationFunctionType
AX = mybir.AxisListType
ALU = mybir.AluOpType

_CACHE = {}


def _blocks(sc):
    """Sigma-block widths: 512s plus one 128-granular remainder."""
    out = [512] * (sc // 512)
    if sc % 512:
        out.append(sc % 512)
    return out


def _emit(tc, aps, sc):
    nc = tc.nc
    ctx = aps["ctx_stack"]
    hst, hsn, w1m, w2t, bvt, hidr, masku, ctx_out = (
        aps["hst"], aps["hsn"], aps["w1m"], aps["w2t"],
        aps["bvt"], aps["hidr"], aps["masku"], aps["ctx"],
    )
    SIGB = _blocks(sc)
    NSIG = len(SIGB)
    TT = sc // 128  # context s-tiles per batch

    def pool(name, bufs, space="SBUF"):
        return ctx.enter_context(tc.tile_pool(name=name, bufs=bufs, space=space))

    p_hst = pool("hst", 5)
    p_w1 = pool("w1", 1)
    p_w2c = pool("w2c", 8)
    p_small = pool("small", 1)
    p_hsn = pool("hsn", BL + 1)
    p_tanh = pool("tanh", 3)
    p_part = pool("part", 2)
    p_eall = pool("eall", 2)
    p_mask = pool("mask", 1)
    p_ctxs = pool("ctxs", 1)
    p_attnT = pool("attnT", BL)
    p_sc = pool("sc", 16)

    pp_pre = pool("ppre", 4, space="PSUM")
    pp_en = pool("pen", 1, space="PSUM")
    pp_tr = pool("ptr", 1, space="PSUM")
    pp_ctx = pool("pctx", 2, space="PSUM")

    # ---------------- setup ----------------
    ident = p_small.tile([1, 1], F32, tag="ident")
    nc.gpsimd.memset(ident[:], 1.0)
    # warm the ACT table set (exp_and_others covers tanh+exp) during DMA wait
    dummy = p_small.tile([1, 1], F32, tag="dummy")
    nc.scalar.activation(dummy[:], ident[:], AF.Tanh)

    # bvt packs [bsum(b1+b2) | vt | eye4 | ones] as (128, 2*HK+5)
    bvt_sb = p_small.tile([128, 2 * HK + 5], F32R, tag="bvt")
    nc.sync.dma_start(bvt_sb[:], bvt[:])
    bsum_sb = bvt_sb[:, 0:HK].bitcast(F32)
    vt_f32 = bvt_sb[:, HK:2 * HK].bitcast(F32)
    eye4 = bvt_sb[0:4, 2 * HK:2 * HK + 4].bitcast(F32)
    ones_col = bvt_sb[:, 2 * HK + 4:2 * HK + 5]  # F32R ones for the v-dot MM
    # hidden (pre-swizzled on host to [p, 4k+b]), fp16 to match the W2 rows
    hid_sb = p_small.tile([128, BL * HK], F16, tag="hidr")
    nc.sync.dma_start(hid_sb[:], hidr[:])

    def load_hst(dst, b, off, W, swdge=False):
        if swdge:
            # one SWDGE DMA: spread over all 16 engines, strictly ordered in
            # the gpsimd queue -- used for the first blocks so the startup
            # stream arrives in consumption order.
            nc.gpsimd.dma_start(
                dst.rearrange("p (k j) -> p k j", k=HK),
                hst[b, :, off:off + W].rearrange("(k p) j -> p k j", p=128),
            )
            return
        # 8 per-k DMAs: each HWDGE dma_start lands on one DMA engine, so
        # splitting is what buys transfer parallelism.
        for k in range(HK):
            nc.sync.dma_start(
                dst[:, W * k:W * (k + 1)],
                hst[b, 128 * k:128 * (k + 1), off:off + W],
            )

    # First hst block + W1 interleaved on the gpsimd queue: SWDGE spreads
    # each DMA across all 16 engines, so these land in consumption order
    # and the first matmul group starts ~1us in.
    hst_first = p_hst.tile([128, HK * SIGB[0]], F16, tag="hst", name="hst_first")
    w1_sb = p_w1.tile([128, HK * H], F16, tag="w1")

    def w1_load(m):
        nc.gpsimd.dma_start(w1_sb[:, HK * 128 * m:HK * 128 * (m + 1)], w1m[m])

    w1_load(0)
    for k in range(4):
        nc.gpsimd.dma_start(
            hst_first[:, SIGB[0] * k:SIGB[0] * (k + 1)],
            hst[0, 128 * k:128 * (k + 1), 0:SIGB[0]],
        )
    w1_load(1)
    for k in range(4, HK):
        nc.gpsimd.dma_start(
            hst_first[:, SIGB[0] * k:SIGB[0] * (k + 1)],
            hst[0, 128 * k:128 * (k + 1), 0:SIGB[0]],
        )
    for m in range(2, HK):
        w1_load(m)

    # all four batch masks in one row
    mask_all = p_mask.tile([1, BL * sc], U8, tag="mask")
    nc.sync.dma_start(mask_all[:], masku[:])

    # W2 rows for the q matmul (scalar queue: parallel issue with hst/w1;
    # ACT is idle until the first tanh which needs qt anyway)
    w2rs = []
    for k in range(HK):
        w2r = p_w2c.tile([128, H], F16, tag="w2c", name=f"w2r{k}")
        nc.scalar.dma_start(w2r[:], w2t[128 * k:128 * (k + 1), :])
        w2rs.append(w2r)

    # qT[h_out, b] = sum_hin W2[h_out, hin] * hidden[b, hin] + b1 + b2.
    # Emitted AFTER the first energy block's matmuls (via q_hook) so the main
    # stream doesn't queue behind the 2 MiB W2 load.
    qt_sb = p_small.tile([128, BL * HK], F32, tag="qt")
    qn_sb = p_small.tile([BL, H], F32, tag="qnat")

    def emit_q():
        for n in range(2):
            pq = pp_ctx.tile([BL, 512], F32, tag="pctx", name=f"pq{n}")
            for k in range(HK):
                nc.tensor.matmul(
                    pq[:],
                    lhsT=hid_sb[:, BL * k:BL * (k + 1)],
                    rhs=w2rs[k][:, 512 * n:512 * (n + 1)],
                    start=(k == 0), stop=(k == HK - 1),
                )
            nc.vector.tensor_copy(qn_sb[:, 512 * n:512 * (n + 1)], pq[:])
        # transpose q to [h_out partition, b] and fold in b1+b2
        ptrq = pp_tr.tile([128, BL * HK], F32, tag="ptr", name="ptrq")
        for m in range(HK):
            nc.tensor.transpose(
                ptrq[:, BL * m:BL * (m + 1)], qn_sb[:, 128 * m:128 * (m + 1)], eye4
            )
        for m in range(HK):
            nc.vector.tensor_scalar_add(
                qt_sb[:, BL * m:BL * (m + 1)], ptrq[:, BL * m:BL * (m + 1)],
                bsum_sb[:, m:m + 1],
            )

    eall_t = {}
    bmax_t = {}
    attnT_t = {}
    rz_t = {}

    # ------------- pass 1: energies for one (batch, sigma-block) -------------
    def p1_block(b, c, first_tile=None, q_hook=None):
        W = SIGB[c]
        off = 512 * c  # block start (all non-final blocks are 512 wide)
        if c == 0:
            eall_t[b] = p_eall.tile([1, sc], F32, tag="eall", name=f"eall{b}")
            bmax_t[b] = p_sc.tile([1, NSIG], F32, tag="bmax", name=f"bmax{b}")
        eall = eall_t[b]
        if first_tile is not None:
            hst_c = first_tile
        else:
            hst_c = p_hst.tile([128, HK * W], F16, tag="hst", name=f"hst_{b}_{c}")
            # the first two loaded blocks ride the ordered SWDGE queue right
            # behind W1/hst_first; later blocks use parallel HWDGE DMAs
            load_hst(hst_c[:], b, off, W, swdge=(b, c) <= (1, 0))
        part = p_part.tile([128, W], F32R, tag="part", name=f"part_{b}_{c}")
        ppres = {}

        def emit_group(m):
            ppre = pp_pre.tile([128, W], F32, tag="ppre", name=f"ppre_{b}_{c}_{m}")
            for k in range(HK):
                nc.tensor.matmul(
                    ppre[:],
                    lhsT=w1_sb[:, (m * HK + k) * 128:(m * HK + k) * 128 + 128],
                    rhs=hst_c[:, W * k:W * (k + 1)],
                    start=(k == 0), stop=(k == HK - 1),
                )
            ppres[m] = ppre

        def emit_act(m):
            th = p_tanh.tile([128, W], F32, tag="tanh", name=f"th_{b}_{c}_{m}")
            nc.scalar.activation(
                th[:], ppres.pop(m), AF.Tanh,
                bias=qt_sb[:, BL * m + b:BL * m + b + 1], scale=1.0,
            )
            # v-weighted accumulate on DVE (frees the PE for the next m-group)
            if m == 0:
                nc.vector.tensor_scalar_mul(part[:], th[:], vt_f32[:, 0:1])
            else:
                nc.vector.scalar_tensor_tensor(
                    part[:], th[:], vt_f32[:, m:m + 1], part[:],
                    op0=ALU.mult, op1=ALU.add,
                )

        if q_hook is not None:
            # first block: matmul groups go out ahead of the q computation so
            # the PE starts on them as soon as W1/hst land; the tanh stream
            # (which reads qt) is emitted after the hook.
            for m in range(4):
                emit_group(m)
            q_hook()
            for m in range(4, HK):
                emit_act(m - 4)
                emit_group(m)
            for m in range(4, HK):
                emit_act(m)
        else:
            for m in range(HK):
                emit_group(m)
                emit_act(m)
        # partition-sum of the v-weighted tanh: single ones-vector matmul
        pen = pp_en.tile([1, W], F32, tag="pen", name=f"pen_{b}_{c}")
        nc.tensor.matmul(pen[:], lhsT=ones_col, rhs=part[:], start=True, stop=True)
        # fold the -1e10 mask into the PSUM->SBUF copy, then per-block max
        # (both hide under the next block's matmuls)
        nc.vector.scalar_tensor_tensor(
            eall[:, off:off + W], mask_all[:, b * sc + off:b * sc + off + W],
            -1e10, pen[:], op0=ALU.mult, op1=ALU.add,
        )
        nc.vector.reduce_max(
            bmax_t[b][:, c:c + 1], eall[:, off:off + W], axis=AX.X
        )

    # ------------- masked softmax, split so it interleaves with pass 1 ------
    def sm_pre(b):
        """exposed part: global max, exp, Z, 1/Z. No PE work."""
        eall = eall_t[b]
        bmax = bmax_t.pop(b)
        negmax = p_sc.tile([1, 1], F32, tag="negmax", name=f"negmax{b}")
        nc.vector.reduce_max(negmax[:], bmax[:], axis=AX.X, negate=True)
        zs = p_sc.tile([1, 1], F32, tag="zs", name=f"zs{b}")
        nc.scalar.activation(eall[:], eall[:], AF.Exp, bias=negmax[:], scale=1.0, accum_out=zs[:])
        rz = p_sc.tile([1, 1], F32, tag="rz", name=f"rz{b}")
        nc.vector.reciprocal(rz[:], zs[:])
        rz_t[b] = rz

    def sm_tr(b):
        """PE part: TT tiny transposes of attn into [s-partition, 1] layout.
        The transpose 'identity' scalar is 1/Z, so attn comes out normalized
        and no post-context rescale is needed."""
        eall = eall_t.pop(b)
        rz = rz_t.pop(b)
        # scale halves separately so the first transposes start half-early
        half = (TT // 2) * 128
        nc.vector.tensor_scalar_mul(eall[:, 0:half], eall[:, 0:half], rz[:])
        nc.vector.tensor_scalar_mul(eall[:, half:sc], eall[:, half:sc], rz[:])
        ptr = pp_tr.tile([128, TT], F32, tag="ptr", name=f"ptr{b}")
        for cc in range(TT):
            nc.tensor.transpose(ptr[:, cc:cc + 1], eall[:, 128 * cc:128 * (cc + 1)], ident[:])
        att = p_attnT.tile([128, TT], BF16, tag="attnT", name=f"attnT{b}")
        nc.vector.tensor_copy(att[:], ptr[:])
        attnT_t[b] = att

    # ------------- pass 2: context, all batches column-packed -------------
    hsn_tiles = {}

    def p2_load(b):
        hsn_b = p_hsn.tile([128, TT * H], BF16, tag="hsn", name=f"hsn_{b}")
        nc.gpsimd.dma_start(
            hsn_b[:].rearrange("p (t h) -> p t h", t=TT),
            hsn[:, b, :].rearrange("(t p) h -> p t h", p=128),
        )
        hsn_tiles[b] = hsn_b

    pcs_t = []

    def p2_mm(batches):
        # batches run concurrently in the PE array via column tiling: each
        # M=1 matmul occupies one 32-column group with its own stream.
        if not pcs_t:
            for n in range(2):
                pc = pp_ctx.tile([128, 512], F32, tag="pctx", name=f"pcn{n}")
                # zero the unwritten rows so the full-width drain copy is
                # clean; runs early on DVE, hidden under pass-1 matmuls
                nc.vector.memset(pc[:], 0.0)
                pcs_t.append(pc)
        for n in range(2):
            for t in range(TT):
                for b in batches:
                    nc.tensor.matmul(
                        pcs_t[n][32 * b:32 * b + 1, :],
                        lhsT=attnT_t[b][:, t:t + 1],
                        rhs=hsn_tiles[b][:, t * H + 512 * n:t * H + 512 * n + 512],
                        start=(t == 0), stop=(t == TT - 1),
                        tile_position=(0, 32 * b),
                        skip_group_check=True,
                    )

    def p2_finish():
        # context is already normalized (attn carried 1/Z): drain the two
        # halves on different engines, then one strided-partition DMA out
        cs_all = p_ctxs.tile([128, H], F32, tag="ctxs")
        nc.vector.tensor_copy(cs_all[:, 0:512], pcs_t[0][:])
        nc.scalar.activation(cs_all[:, 512:H], pcs_t[1][:], AF.Copy)
        nc.sync.dma_start(ctx_out[:], cs_all[::32, :])
        for b in range(BL):
            attnT_t.pop(b)
            hsn_tiles.pop(b)

    # ------------- schedule -------------
    # sm(b) pieces interleave into the middle of batch b+1's PE stream so the
    # softmax chain latency hides behind matmuls; the packed context phase
    # runs once at the end when all four attn vectors are ready.
    p1_block(0, 0, first_tile=hst_first, q_hook=emit_q)
    for c in range(1, NSIG):
        p1_block(0, c)
    for b in range(1, BL):
        p1_block(b, 0)
        sm_pre(b - 1)
        p2_load(b - 1)  # hsn loads stay clear of the startup DMA burst
        if NSIG > 1:
            p1_block(b, 1)
        sm_tr(b - 1)
        for c in range(2, NSIG):
            p1_block(b, c)
    p2_load(BL - 1)
    # batches 0..BL-2 run their (packed) context matmuls while the last
    # batch's softmax chain drains; the last batch joins afterwards.
    p2_mm(list(range(BL - 1)))
    sm_pre(BL - 1)
    sm_tr(BL - 1)
    p2_mm([BL - 1])
    p2_finish()


def build_program(sc):
    if sc in _CACHE:
        return _CACHE[sc]
    nc = bacc.Bacc("TRN2", target_bir_lowering=False, debug=False, enable_asserts=False)
    aps = {
        "hst": nc.dram_tensor("hst", (BL, H, sc), F16, kind="ExternalInput").ap(),
        "hsn": nc.dram_tensor("hsn", (sc, BL, H), BF16, kind="ExternalInput").ap(),
        "w1m": nc.dram_tensor("w1m", (HK, 128, H), F16, kind="ExternalInput").ap(),
        "w2t": nc.dram_tensor("w2t", (H, H), F16, kind="ExternalInput").ap(),
        "bvt": nc.dram_tensor("bvt", (128, 2 * HK + 5), F32R, kind="ExternalInput").ap(),
        "hidr": nc.dram_tensor("hidr", (128, BL * HK), F16, kind="ExternalInput").ap(),
        "masku": nc.dram_tensor("masku", (1, BL * sc), U8, kind="ExternalInput").ap(),
        "ctx": nc.dram_tensor("ctx", (BL, H), F32, kind="ExternalOutput").ap(),
    }
    with tile.TileContext(nc) as tc:
        with ExitStack() as stack:
            aps["ctx_stack"] = stack
            _emit(tc, aps, sc)
    nc.compile()
    _CACHE[sc] = nc
    return nc


def prep_in_maps(inputs):
    hidden = np.ascontiguousarray(np.asarray(inputs["hidden"], dtype=np.float32))
    hs = np.asarray(inputs["hidden_sequence"], dtype=np.float32)
    masks = np.asarray(inputs["input_masks"]).astype(bool)
    w1 = np.asarray(inputs["W1"], dtype=np.float32)
    w2t = np.ascontiguousarray(np.asarray(inputs["W2"], dtype=np.float32).T.astype(np.float16))
    b1 = np.asarray(inputs["b1"], dtype=np.float32)
    b2 = np.asarray(inputs["b2"], dtype=np.float32)
    v = np.asarray(inputs["v"], dtype=np.float32)

    # ---- mask compaction: per batch, gather unmasked rows ----
    counts = masks.sum(axis=0)
    if counts.min() == 0:
        # all-masked batch: softmax degenerates to uniform over the FULL S,
        # so compaction would change the result. Fall back to identity order.
        sc = S
        idx_list = [np.arange(S)] * B
    else:
        sc = max(128, int(-(-int(counts.max()) // 128) * 128))
        idx_list = [np.nonzero(masks[:, b])[0] for b in range(B)]

    # compact hs and the inverted mask
    hs_c = np.zeros((B, sc, H), dtype=np.float32)
    minv = np.ones((B, sc), dtype=np.uint8)
    for b in range(B):
        idx = idx_list[b]
        n = len(idx)
        hs_c[b, :n] = hs[idx[:n] if n else idx, b, :]
        if sc == S and counts.min() == 0:
            # identity order: apply the original mask
            hs_c[b] = hs[:, b, :]
            minv[b] = (~masks[:, b]).astype(np.uint8)
        else:
            minv[b, :n] = 0

    # [bsum | vt | eye4 | ones] packed as (128, 2*HK+5)
    ey = np.zeros((128, 4), dtype=np.float32)
    ey[0:4, 0:4] = np.eye(4, dtype=np.float32)
    ones = np.ones((128, 1), dtype=np.float32)
    bvt_base = np.concatenate(
        [(b1 + b2).reshape(HK, 128).T, v.reshape(HK, 128).T, ey, ones], axis=1
    )

    # W1 packed m-major: w1m[m][p, k*128+j] = W1[128m+j, 128k+p]
    w1m = np.ascontiguousarray(
        w1.reshape(HK, 128, HK, 128).transpose(0, 3, 2, 1).reshape(HK, 128, H)
    ).astype(np.float16)

    in_maps = []
    for ci in range(NCORES):
        g = slice(BL * ci, BL * (ci + 1))
        blk = hs_c[g]  # (BL, sc, H)
        hg = hidden[0, g, :]  # (BL, H)
        # hidr[p, BL*k + b] = hidden[b, 128k + p]
        hidr = np.ascontiguousarray(
            hg.T.reshape(HK, 128, BL).transpose(1, 0, 2).reshape(128, HK * BL).astype(np.float16)
        )
        in_maps.append({
            "hst": np.ascontiguousarray(blk.transpose(0, 2, 1)).astype(np.float16),
            "hsn": np.ascontiguousarray(blk.transpose(1, 0, 2).astype(ml_dtypes.bfloat16)),
            "w1m": w1m,
            "w2t": w2t,
            "bvt": np.ascontiguousarray(bvt_base),
            "hidr": hidr,
            "masku": np.ascontiguousarray(minv[g]).reshape(1, BL * sc),
        })
    return sc, in_maps


def _gather(res):
    out = np.concatenate([res.results[i]["ctx"] for i in range(NCORES)], axis=0)
    return out[None].astype(np.float32)


def kernel(**inputs):
    sc, in_maps = prep_in_maps(inputs)
    nc = build_program(sc)
    res = bass_utils.run_bass_kernel_spmd(nc, in_maps, list(range(NCORES)))
    return _gather(res)


def run_traced(inputs, trace=True):
    """test.py entry: returns (BassKernelResults, full output)."""
    sc, in_maps = prep_in_maps(inputs)
    nc = build_program(sc)
    res = bass_utils.run_bass_kernel_spmd(
        nc, in_maps, list(range(NCORES)), trace=trace
    )
    return res, _gather(res)


if __name__ == "__main__":
    build_program(1152)
    print("program built OK")
